# revision 1
# baseline (speedup 1.0000x reference)
"""Bahdanau attention kernel for 8 Trainium2 NeuronCores.

Strategy (single SPMD launch, one NEFF on all 8 cores):
  - Scores phase is tensor-parallel over the hidden dim H: core i owns
    h-slice [256*i, 256*(i+1)).  Each core computes
      q_projT[h_i, b], v_projT[h_i, s]  (fp32r matmuls, fp32 accumulate)
      tanh(v_projT + q_projT[b]) via ScalarE with the per-partition bias port
      partial scores[b, s] = V_w[h_i] . tanh(...)  via M=16 zero-embedded
        column matmuls into (16, 512) PSUM banks.
    The s axis is processed in two halves so the tanh pipeline starts while
    the second half of v_projT is still accumulating.
  - Partial scores are ReduceScatter-summed across the 8 cores: rank i's
    chunk is exactly score rows {2i, 2i+1} — its two batches.
  - Context phase is data-parallel over batch: softmax (ScalarE exp with
    accumulate), alphas transposed via PE, context[b] = alphasT.T @ values[b]
    (fp32r), streamed from HBM with deep prefetch.
Host side only reshapes/slices/transposes inputs (sharding layout) and
concatenates the per-core outputs.
"""

import sys

sys.path.insert(0, "/opt/trn_rl_repo")

import numpy as np

import concourse.bass as bass  # noqa: F401  (registers AP machinery)
import concourse.tile as tile
from concourse import bacc, mybir
from concourse.bass_utils import run_bass_kernel_spmd
from concourse.masks import make_identity

H = 2048
B = 16
S = 2048
NC = 8
P = 128
HLOC = H // NC  # 256
KT = H // P  # 16 contraction tiles
ST = S // P  # 16 s tiles
NT = S // 512  # 4 free-dim slices of 512

F32 = mybir.dt.float32
F32R = mybir.dt.float32r
F16 = mybir.dt.float16
BF16 = mybir.dt.bfloat16

N_PRE = 28  # context-values tiles prefetched (== vlp bufs)

_TRACE = False
LAST_EXEC_NS = None

_NC_CACHE = []


def _build_module():
    nc = bacc.Bacc("TRN2", target_bir_lowering=False, debug=False, num_devices=NC)

    v0t = nc.dram_tensor("v0t", [H, S], F16, kind="ExternalInput")  # values[0].T
    w2t = nc.dram_tensor("w2t", [H, HLOC], F16, kind="ExternalInput")  # W2[h_i].T
    w1t = nc.dram_tensor("w1t", [H, HLOC], F32, kind="ExternalInput")  # W1[h_i].T
    qt = nc.dram_tensor("qt", [H, B], F32, kind="ExternalInput")  # q.T
    b12 = nc.dram_tensor("b12", [P, 2, 2], F32, kind="ExternalInput")  # biases
    vwe = nc.dram_tensor("vwe", [P, 2, B, B], F16, kind="ExternalInput")
    vals = nc.dram_tensor("vals", [2, S, H], F16, kind="ExternalInput")
    ctx_o = nc.dram_tensor("ctx", [2, H], F32, kind="ExternalOutput")
    alp_o = nc.dram_tensor("alp", [2, S], F32, kind="ExternalOutput")

    with tile.TileContext(nc) as tc:
        with tc.tile_pool(name="const", bufs=1) as const:
            # ---- resident SBUF state -------------------------------------
            w2s = const.tile([P, KT, HLOC], F16)
            nc.sync.dma_start(
                out=w2s, in_=w2t[:, :].rearrange("(t p) m -> p t m", p=P)
            )
            vwes = const.tile([P, 2, B, B], F16)
            nc.gpsimd.dma_start(out=vwes, in_=vwe[:, :, :, :])
            b12s = const.tile([P, 2, 2], F32)
            nc.gpsimd.dma_start(out=b12s, in_=b12[:, :, :])
            ident = const.tile([P, P], F32)
            make_identity(nc, ident[:, :])

            bsum = const.tile([P, 2], F32)
            nc.vector.tensor_add(out=bsum, in0=b12s[:, :, 0], in1=b12s[:, :, 1])

            qpt = const.tile([P, 2, B], F32)  # q_projT + bias
            vps = const.tile([P, 2, S], F32)  # v_projT (SBUF resident)
            scs = const.tile([B, S], F32)  # partial scores
            msc = const.tile([2, S], F32)  # my 2 rows of summed scores
            alp = const.tile([2, S], F32)  # alphas
            mx = const.tile([2, 1], F32)
            nmx = const.tile([2, 1], F32)
            ssum = const.tile([2, 1], F32)
            rec = const.tile([2, 1], F32)
            alT = const.tile([P, ST, 2], F16)  # alphas transposed
            ctxs = const.tile([1, H], F32)
            wu = const.tile([P, 512], BF16)  # PE warm-up junk

            with tc.tile_pool(name="pha", bufs=1) as pha:
                w1s = pha.tile([P, KT, HLOC], F32)
                nc.gpsimd.dma_start(
                    out=w1s, in_=w1t[:, :].rearrange("(t p) m -> p t m", p=P)
                )
                qts = pha.tile([P, KT, B], F32)
                nc.sync.dma_start(
                    out=qts, in_=qt[:, :].rearrange("(t p) b -> p t b", p=P)
                )

                # ---- PE warm-up: dummy matmuls to lift HAM to 2.4 GHz ----
                nc.vector.memset(wu[:, :], 0.0)
                with tc.tile_pool(name="psw", bufs=1, space="PSUM") as psw:
                    wup = psw.tile([P, 512], F32, tag="wup", name="wup")
                    n_wu = 16
                    for i in range(n_wu):
                        nc.tensor.matmul(
                            wup[:, :], wu[:, 0:P], wu[:, :],
                            start=(i == 0), stop=(i == n_wu - 1),
                        )
                    nc.vector.tensor_copy(out=wu[:, 0:P], in_=wup[:, 0:P])

                # ---- phase A: q_projT (exact fp32; needed by phase C) ----
                with tc.tile_pool(name="psa", bufs=2, space="PSUM") as psa:
                    for m in range(2):
                        qp_ps = psa.tile([P, B], F32, tag="qp", name="qp")
                        for kt in range(KT):
                            nc.tensor.matmul(
                                qp_ps[:, :],
                                w1s[:, kt, m * P : (m + 1) * P],
                                qts[:, kt, :],
                                start=(kt == 0),
                                stop=(kt == KT - 1),
                            )
                        nc.vector.tensor_scalar_add(
                            out=qpt[:, m, :], in0=qp_ps[:, :],
                            scalar1=bsum[:, m : m + 1],
                        )

                # ---- phase B: v_projT (fp32r) ----------------------------
                with (
                    tc.tile_pool(name="psb", bufs=1, space="PSUM") as psb,
                    tc.tile_pool(name="v0p", bufs=3) as v0p,
                ):
                    vpp = [
                        [
                            psb.tile(
                                [P, 512], F32,
                                name=f"vp{m}{nt}", tag=f"vp{m}{nt}",
                            )
                            for nt in range(NT)
                        ]
                        for m in range(2)
                    ]
                    for kt in range(KT):
                        rv = v0p.tile([P, S], F16, tag="rv", name="rv")
                        nc.sync.dma_start(
                            out=rv, in_=v0t[kt * P : (kt + 1) * P, :]
                        )
                        for m in range(2):
                            for nt in range(NT):
                                nc.tensor.matmul(
                                    vpp[m][nt][:, :],
                                    w2s[:, kt, m * P : (m + 1) * P],
                                    rv[:, nt * 512 : (nt + 1) * 512],
                                    start=(kt == 0),
                                    stop=(kt == KT - 1),
                                )
                    for m in range(2):
                        for nt in range(NT):
                            nc.vector.tensor_copy(
                                out=vps[:, m, nt * 512 : (nt + 1) * 512],
                                in_=vpp[m][nt],
                            )

            # context values: start prefetching as soon as DMA queues allow
            vlp_cm = tc.tile_pool(name="vlp", bufs=N_PRE)
            vlp = vlp_cm.__enter__()
            vts = {}
            for j in range(N_PRE):
                b, kt = divmod(j, KT)
                vt = vlp.tile([P, H], F16, tag="vt", name="vt")
                nc.sync.dma_start(
                    out=vt, in_=vals[b, kt * P : (kt + 1) * P, :]
                )
                vts[(b, kt)] = vt

            # ---- phase C: tanh + partial scores --------------------------
            with (
                tc.tile_pool(name="psc", bufs=1, space="PSUM") as psc,
                tc.tile_pool(name="thp", bufs=2) as thp,
            ):
                scps = [
                    psc.tile([B, 512], F32, name=f"sc{nt}", tag=f"sc{nt}")
                    for nt in range(NT)
                ]
                for b in range(B):
                    for m in range(2):
                        th = thp.tile([P, S], F16, tag="th", name="th")
                        nc.scalar.activation(
                            out=th[:, :],
                            in_=vps[:, m, :],
                            func=mybir.ActivationFunctionType.Tanh,
                            bias=qpt[:, m, b : b + 1],
                            scale=1.0,
                        )
                        for nt in range(NT):
                            nc.tensor.matmul(
                                scps[nt][:, :],
                                vwes[:, m, b, :],
                                th[:, nt * 512 : (nt + 1) * 512],
                                start=(b == 0 and m == 0),
                                stop=(b == B - 1 and m == 1),
                            )
                for nt in range(NT):
                    nc.vector.tensor_copy(
                        out=scs[:, nt * 512 : (nt + 1) * 512], in_=scps[nt][:, :]
                    )

            # ---- keep PE warm through the collective window --------------
            with tc.tile_pool(name="psw2", bufs=1, space="PSUM") as psw2:
                wup2 = psw2.tile([P, 512], F32, tag="wup2", name="wup2")
                n_wu2 = 100
                for i in range(n_wu2):
                    nc.tensor.matmul(
                        wup2[:, :], wu[:, 0:P], wu[:, :],
                        start=(i == 0), stop=(i == n_wu2 - 1),
                    )
                nc.vector.tensor_copy(out=wu[:, 0:P], in_=wup2[:, 0:P])

            # ---- phase D: ReduceScatter -> my 2 summed score rows --------
            with tc.tile_pool(name="drp", bufs=1, space="DRAM") as drp:
                arin = drp.tile([B, S], F32, name="arin")
                arout = drp.tile([2, S], F32, name="arout")
                nc.sync.dma_start(out=arin[:, :], in_=scs[:, :])
                nc.gpsimd.collective_compute(
                    "ReduceScatter",
                    mybir.AluOpType.add,
                    replica_groups=[list(range(NC))],
                    ins=[arin.opt()],
                    outs=[arout.opt()],
                )
                nc.sync.dma_start(out=msc[:, :], in_=arout[:, :])

            # ---- phase E: softmax ----------------------------------------
            with tc.tile_pool(name="psef", bufs=2, space="PSUM") as psef:
                nc.vector.tensor_reduce(
                    out=mx, in_=msc[:, :], axis=mybir.AxisListType.X,
                    op=mybir.AluOpType.max,
                )
                nc.vector.tensor_scalar_mul(out=nmx, in0=mx, scalar1=-1.0)
                nc.scalar.activation(
                    out=alp[:, :],
                    in_=msc[:, :],
                    func=mybir.ActivationFunctionType.Exp,
                    bias=nmx[:, 0:1],
                    scale=1.0,
                    accum_out=ssum[:, 0:1],
                )
                nc.vector.reciprocal(out=rec, in_=ssum)
                nc.vector.tensor_scalar_mul(
                    out=alp[:, :], in0=alp[:, :], scalar1=rec[:, 0:1]
                )
                nc.sync.dma_start(out=alp_o[:, :], in_=alp[:, :])

                # ---- phase F: alphas transposed --------------------------
                for j in range(ST):
                    tp_ = psef.tile([P, 2], F32, tag="tr", name="tp", bufs=4)
                    nc.tensor.transpose(
                        tp_[:, :], alp[:, j * P : (j + 1) * P], ident[0:2, 0:2]
                    )
                    nc.vector.tensor_copy(out=alT[:, j, :], in_=tp_)

            # ---- phase G: context = alphasT.T @ values[b] ----------------
            with tc.tile_pool(name="psg", bufs=1, space="PSUM") as psg:
                cps = [
                    [
                        psg.tile([1, 512], F32, name=f"cx{b}{nt}", tag=f"cx{b}{nt}")
                        for nt in range(NT)
                    ]
                    for b in range(2)
                ]
                for b in range(2):
                    for kt in range(KT):
                        if (b, kt) in vts:
                            vt = vts[(b, kt)]
                        else:
                            vt = vlp.tile([P, H], F16, tag="vt", name="vt")
                            nc.sync.dma_start(
                                out=vt,
                                in_=vals[b, kt * P : (kt + 1) * P, :],
                            )
                        for nt in range(NT):
                            nc.tensor.matmul(
                                cps[b][nt][:, :],
                                alT[:, kt, b : b + 1],
                                vt[:, nt * 512 : (nt + 1) * 512],
                                start=(kt == 0),
                                stop=(kt == KT - 1),
                            )
                for b in range(2):
                    for nt in range(NT):
                        nc.vector.tensor_copy(
                            out=ctxs[:, nt * 512 : (nt + 1) * 512],
                            in_=cps[b][nt][:, :],
                        )
                    nc.sync.dma_start(out=ctx_o[b : b + 1, :], in_=ctxs[:, :])
            vlp_cm.__exit__(None, None, None)

    nc.compile()
    return nc


def _get_module():
    if not _NC_CACHE:
        _NC_CACHE.append(_build_module())
    return _NC_CACHE[0]


def kernel(query, values, mask=None, W1_w=None, W1_b=None, W2_w=None, W2_b=None,
           V_w=None, V_b=None):
    global LAST_EXEC_NS
    query = np.ascontiguousarray(np.asarray(query, dtype=np.float32))
    values = np.ascontiguousarray(np.asarray(values, dtype=np.float32))
    W1_w = np.asarray(W1_w, dtype=np.float32)
    W1_b = np.asarray(W1_b, dtype=np.float32)
    W2_w = np.asarray(W2_w, dtype=np.float32)
    W2_b = np.asarray(W2_b, dtype=np.float32)
    V_w = np.asarray(V_w, dtype=np.float32)

    q = query[0][:, -1, :]  # (B, H)
    v0t = np.ascontiguousarray(values[0].T.astype(np.float16))  # (H, S)
    qt = np.ascontiguousarray(q.T)  # (H, B)

    in_maps = []
    for i in range(NC):
        hsl = slice(HLOC * i, HLOC * (i + 1))
        w2t_i = np.ascontiguousarray(W2_w[hsl, :].T.astype(np.float16))  # (H, HLOC)
        w1t_i = np.ascontiguousarray(W1_w[hsl, :].T)
        b12_i = np.zeros((P, 2, 2), np.float32)
        b12_i[:, :, 0] = W1_b[hsl].reshape(2, P).T
        b12_i[:, :, 1] = W2_b[hsl].reshape(2, P).T
        vwl = V_w[hsl].astype(np.float16).reshape(2, P)  # [m, p]
        vwe_i = np.zeros((P, 2, B, B), np.float16)
        for bb in range(B):
            vwe_i[:, :, bb, bb] = vwl.T
        in_maps.append(
            {
                "v0t": v0t,
                "w2t": w2t_i,
                "w1t": w1t_i,
                "qt": qt,
                "b12": b12_i,
                "vwe": vwe_i,
                "vals": np.ascontiguousarray(values[2 * i : 2 * i + 2].astype(np.float16)),
            }
        )

    nc = _get_module()
    res = run_bass_kernel_spmd(
        nc, in_maps, core_ids=list(range(NC)), trace=_TRACE
    )
    LAST_EXEC_NS = res.exec_time_ns

    ctx = np.concatenate([res.results[i]["ctx"] for i in range(NC)], axis=0)
    alps = np.concatenate([res.results[i]["alp"] for i in range(NC)], axis=0)
    return ctx.reshape(B, 1, H), alps.reshape(B, 1, S)



# revision 9
# speedup vs baseline: 1.2722x; 1.2722x over previous
"""Bahdanau attention kernel for 8 Trainium2 NeuronCores.

Strategy (single SPMD launch, one NEFF on all 8 cores):
  - Scores phase is tensor-parallel over the hidden dim H: core i owns
    h-slice [256*i, 256*(i+1)).  v_projT is computed in two s-halves so
    the ScalarE tanh pipeline starts as soon as the first half of
    values[0].T has streamed in.  Weights are fp16, DMA-ordered so v0t
    streams at full rate before the context values prefetch.
  - Partial scores are exchanged with an AllToAll (fp16, 64KB) instead
    of a ReduceScatter; the 8 shards are summed on the PE with a small
    selector matmul, in BOTH layouts: [2, S] for the alphas output and
    [S-partition, 2] (transposed) feeding exp directly into the fp16
    alphasT tile for the context matmul (no PE transposes needed).
  - exp without max-subtraction (scores are O(3)), normalization folded
    into the PSUM->SBUF copy of the context rows.
  - Context phase is 2-way column-tiled (tile_position): the two
    batches' accumulation chains run concurrently on separate PE column
    groups.  A junk-MM chain keeps the PE clock warm through the
    collective window.
Host side only reshapes/slices/transposes inputs (sharding layout) and
concatenates the per-core outputs.
"""

import sys

sys.path.insert(0, "/opt/trn_rl_repo")

import numpy as np

import concourse.bass as bass  # noqa: F401  (registers AP machinery)
import concourse.tile as tile
from concourse import bacc, mybir
from concourse.bass_utils import run_bass_kernel_spmd

H = 2048
B = 16
S = 2048
NC = 8
P = 128
HLOC = H // NC  # 256
KT = H // P  # 16 contraction tiles
SH = S // 2  # 1024: s-half size
NT = S // 512  # 4 free-dim slices of 512

F32 = mybir.dt.float32
F16 = mybir.dt.float16
BF16 = mybir.dt.bfloat16

N_PRE = 32  # context-values tiles prefetched (all of them)
N_WARM2 = 110  # keep-warm MMs (N=128) during the collective window

_TRACE = False
LAST_EXEC_NS = None

_NC_CACHE = []


def _build_module():
    nc = bacc.Bacc("TRN2", target_bir_lowering=False, debug=False, num_devices=NC)

    v0t = nc.dram_tensor("v0t", [H, S], F16, kind="ExternalInput")  # values[0].T
    w2t = nc.dram_tensor("w2t", [H, HLOC], F16, kind="ExternalInput")  # W2[h_i].T
    w1t = nc.dram_tensor("w1t", [H, HLOC], F16, kind="ExternalInput")  # W1[h_i].T
    qt = nc.dram_tensor("qt", [H, B], F16, kind="ExternalInput")  # q.T
    b12 = nc.dram_tensor("b12", [P, 2, 2], F32, kind="ExternalInput")  # biases
    vwe = nc.dram_tensor("vwe", [P, 2, B, B], F16, kind="ExternalInput")
    sel = nc.dram_tensor("sel", [B, 2], F16, kind="ExternalInput")  # shard-sum sel
    vals = nc.dram_tensor("vals", [2, S, H], F16, kind="ExternalInput")
    ctx_o = nc.dram_tensor("ctx", [2, H], F32, kind="ExternalOutput")
    alp_o = nc.dram_tensor("alp", [2, S], F32, kind="ExternalOutput")

    with tile.TileContext(nc) as tc:
        with tc.tile_pool(name="const", bufs=1) as const:
            # ---- resident SBUF state -------------------------------------
            w2s = const.tile([P, KT, HLOC], F16)
            nc.sync.dma_start(
                out=w2s, in_=w2t[:, :].rearrange("(t p) m -> p t m", p=P)
            )
            vwes = const.tile([P, 2, B, B], F16)
            nc.gpsimd.dma_start(out=vwes, in_=vwe[:, :, :, :])
            b12s = const.tile([P, 2, 2], F32)
            nc.gpsimd.dma_start(out=b12s, in_=b12[:, :, :])
            sels = const.tile([B, 2], F16)
            nc.gpsimd.dma_start(out=sels, in_=sel[:, :])

            bsum = const.tile([P, 2], F32)
            nc.vector.tensor_add(out=bsum, in0=b12s[:, :, 0], in1=b12s[:, :, 1])

            qpt = const.tile([P, 2, B], F32)  # q_projT + bias
            vps = const.tile([P, 2, S], F32)  # v_projT (SBUF resident)
            scs = const.tile([B, S], F16)  # partial scores (fp16 for A2A)
            a2as = const.tile([B, S], F16)  # A2A result: 8 stacked shards
            alT = const.tile([P, KT, 2], F16)  # exp(scores) transposed
            alp = const.tile([2, S], F32)  # alphas ([2, S] output path)
            ssum = const.tile([2, 1], F32)
            rec = const.tile([2, 1], F32)
            rec32 = const.tile([33, 1], F32)  # rec[b] scattered to partition 32b
            ctxs = const.tile([33, H], F32)  # rows 0 and 32 used
            wu = const.tile([P, 512], BF16)  # PE warm-up junk

            with tc.tile_pool(name="pha", bufs=1) as pha:
                w1s = pha.tile([P, KT, HLOC], F16)
                nc.sync.dma_start(
                    out=w1s, in_=w1t[:, :].rearrange("(t p) m -> p t m", p=P)
                )
                qts = pha.tile([P, KT, B], F16)
                nc.sync.dma_start(
                    out=qts, in_=qt[:, :].rearrange("(t p) b -> p t b", p=P)
                )

                # ---- phase A: q_projT ------------------------------------
                with tc.tile_pool(name="psa", bufs=2, space="PSUM") as psa:
                    for m in range(2):
                        qp_ps = psa.tile([P, B], F32, tag="qp", name="qp")
                        for kt in range(KT):
                            nc.tensor.matmul(
                                qp_ps[:, :],
                                w1s[:, kt, m * P : (m + 1) * P],
                                qts[:, kt, :],
                                start=(kt == 0),
                                stop=(kt == KT - 1),
                            )
                        nc.vector.tensor_scalar_add(
                            out=qpt[:, m, :], in0=qp_ps[:, :],
                            scalar1=bsum[:, m : m + 1],
                        )

                # ---- PE warm-up: dummy matmuls ---------------------------
                nc.vector.memset(wu[:, :], 0.0)
                with tc.tile_pool(name="psw", bufs=1, space="PSUM") as psw:
                    wup = psw.tile([P, 512], F32, tag="wup", name="wup")
                    n_wu = 12
                    for i in range(n_wu):
                        nc.tensor.matmul(
                            wup[:, :], wu[:, 0:P], wu[:, :],
                            start=(i == 0), stop=(i == n_wu - 1),
                        )
                    nc.vector.tensor_copy(out=wu[:, 0:P], in_=wup[:, 0:P])

            # ---- phase B: v_projT (fp16), streamed in two s-halves -------
            with (
                tc.tile_pool(name="psb", bufs=1, space="PSUM") as psb,
                tc.tile_pool(name="v0p", bufs=10) as v0p,
            ):
                for half in range(2):
                    vpp = [
                        [
                            psb.tile(
                                [P, 512], F32,
                                name=f"vp{m}{nt2}", tag=f"vp{m}{nt2}",
                            )
                            for nt2 in range(2)
                        ]
                        for m in range(2)
                    ]
                    for kt in range(KT):
                        rv = v0p.tile([P, SH], F16, tag="rv", name="rv")
                        nc.sync.dma_start(
                            out=rv,
                            in_=v0t[kt * P : (kt + 1) * P, half * SH : (half + 1) * SH],
                        )
                        for m in range(2):
                            for nt2 in range(2):
                                nc.tensor.matmul(
                                    vpp[m][nt2][:, :],
                                    w2s[:, kt, m * P : (m + 1) * P],
                                    rv[:, nt2 * 512 : (nt2 + 1) * 512],
                                    start=(kt == 0),
                                    stop=(kt == KT - 1),
                                )
                    for m in range(2):
                        for nt2 in range(2):
                            nc.vector.tensor_copy(
                                out=vps[
                                    :, m,
                                    half * SH + nt2 * 512 : half * SH + (nt2 + 1) * 512,
                                ],
                                in_=vpp[m][nt2],
                            )

            # context values: prefetch after v0t is fully queued
            vlp_cm = tc.tile_pool(name="vlp", bufs=N_PRE)
            vlp = vlp_cm.__enter__()
            vts = {}
            for j in range(N_PRE):
                b, kt = divmod(j, KT)
                vt = vlp.tile([P, H], F16, tag="vt", name="vt")
                nc.sync.dma_start(
                    out=vt, in_=vals[b, kt * P : (kt + 1) * P, :]
                )
                vts[(b, kt)] = vt

            # ---- phase C: tanh + partial scores (per s-half) -------------
            with (
                tc.tile_pool(name="psc", bufs=1, space="PSUM") as psc,
                tc.tile_pool(name="thp", bufs=10) as thp,
            ):
                scps = psc.tile([B, S], F32, name="scps", tag="scps")
                for half in range(2):
                    for b in range(B):
                        for m in range(2):
                            th = thp.tile([P, SH], F16, tag="th", name="th")
                            nc.scalar.activation(
                                out=th[:, :],
                                in_=vps[:, m, half * SH : (half + 1) * SH],
                                func=mybir.ActivationFunctionType.Tanh,
                                bias=qpt[:, m, b : b + 1],
                                scale=1.0,
                            )
                            for nt2 in range(2):
                                nc.tensor.matmul(
                                    scps[
                                        :,
                                        half * SH + nt2 * 512 : half * SH + (nt2 + 1) * 512,
                                    ],
                                    vwes[:, m, b, :],
                                    th[:, nt2 * 512 : (nt2 + 1) * 512],
                                    start=(b == 0 and m == 0),
                                    stop=(b == B - 1 and m == 1),
                                )
                nc.vector.tensor_copy(out=scs[:, :], in_=scps[:, :])

            # ---- phase D: AllToAll exchange of partial scores ------------
            with tc.tile_pool(name="drp", bufs=1, space="DRAM") as drp:
                arin = drp.tile([B, S], F16, name="arin")
                arout = drp.tile([B, S], F16, name="arout")
                nc.sync.dma_start(out=arin[:, :], in_=scs[:, :])

                # keep PE warm through the collective window
                with tc.tile_pool(name="psw2", bufs=1, space="PSUM") as psw2:
                    wup2 = psw2.tile([P, P], F32, tag="wup2", name="wup2")
                    for i in range(N_WARM2):
                        nc.tensor.matmul(
                            wup2[:, :], wu[:, 0:P], wu[:, 0:P],
                            start=(i == 0), stop=(i == N_WARM2 - 1),
                        )
                    nc.vector.tensor_copy(out=wu[:, 0:P], in_=wup2[:, :])

                nc.gpsimd.collective_compute(
                    "AllToAll",
                    mybir.AluOpType.bypass,
                    replica_groups=[list(range(NC))],
                    ins=[arin.opt()],
                    outs=[arout.opt()],
                )
                nc.sync.dma_start(out=a2as[:, :], in_=arout[:, :])

            # ---- phase E: shard-sum on PE + softmax ----------------------
            # transposed layout first (critical path -> context)
            with tc.tile_pool(name="psqt", bufs=1, space="PSUM") as psqt:
                scpT = psqt.tile([P, KT, 2], F32, name="scpT", tag="scpT")
                for j in range(KT):
                    nc.tensor.matmul(
                        scpT[:, j, :],
                        a2as[:, j * P : (j + 1) * P],
                        sels[:, :],
                        start=True,
                        stop=True,
                    )
                # unnormalized exp(scores).T straight into the fp16 tile
                nc.scalar.activation(
                    out=alT[:, :, :],
                    in_=scpT[:, :, :],
                    func=mybir.ActivationFunctionType.Exp,
                    scale=1.0,
                )

            with tc.tile_pool(name="psq2", bufs=1, space="PSUM") as psq2:
                scp2 = psq2.tile([2, S], F32, name="scp2", tag="scp2")
                for nt in range(NT):
                    nc.tensor.matmul(
                        scp2[:, nt * 512 : (nt + 1) * 512],
                        sels[:, :],
                        a2as[:, nt * 512 : (nt + 1) * 512],
                        start=True,
                        stop=True,
                    )
                # alphas output path (off the context critical path)
                nc.scalar.activation(
                    out=alp[:, :],
                    in_=scp2[:, :],
                    func=mybir.ActivationFunctionType.Exp,
                    scale=1.0,
                    accum_out=ssum[:, 0:1],
                )
                nc.vector.reciprocal(out=rec, in_=ssum)
                nc.gpsimd.dma_start(out=rec32[0:33:32, 0:1], in_=rec[:, 0:1])
                nc.vector.tensor_scalar_mul(
                    out=alp[:, :], in0=alp[:, :], scalar1=rec[:, 0:1]
                )
                nc.gpsimd.dma_start(out=alp_o[:, :], in_=alp[:, :])

                # ---- phase G: context, 2-way column-tiled ----------------
                with tc.tile_pool(name="psg", bufs=1, space="PSUM") as psg:
                    cps = psg.tile([P, S], F32, name="cps", tag="cps")
                    for kt in range(KT):
                        for b in range(2):
                            vt = vts[(b, kt)]
                            for nt in range(NT):
                                nc.tensor.matmul(
                                    cps[32 * b : 32 * b + 1, nt * 512 : (nt + 1) * 512],
                                    alT[:, kt, b : b + 1],
                                    vt[:, nt * 512 : (nt + 1) * 512],
                                    tile_position=(0, 32 * b),
                                    start=(kt == 0),
                                    stop=(kt == KT - 1),
                                )
                    # normalize while copying PSUM -> SBUF
                    for b in range(2):
                        nc.vector.tensor_scalar_mul(
                            out=ctxs[32 * b : 32 * b + 1, :],
                            in0=cps[32 * b : 32 * b + 1, :],
                            scalar1=rec32[32 * b : 32 * b + 1, 0:1],
                        )
                    nc.sync.dma_start(out=ctx_o[:, :], in_=ctxs[0:33:32, :])
            vlp_cm.__exit__(None, None, None)

    nc.compile()
    return nc


def _get_module():
    if not _NC_CACHE:
        _NC_CACHE.append(_build_module())
    return _NC_CACHE[0]


def kernel(query, values, mask=None, W1_w=None, W1_b=None, W2_w=None, W2_b=None,
           V_w=None, V_b=None):
    global LAST_EXEC_NS
    query = np.ascontiguousarray(np.asarray(query, dtype=np.float32))
    values = np.ascontiguousarray(np.asarray(values, dtype=np.float32))
    W1_w = np.asarray(W1_w, dtype=np.float32)
    W1_b = np.asarray(W1_b, dtype=np.float32)
    W2_w = np.asarray(W2_w, dtype=np.float32)
    W2_b = np.asarray(W2_b, dtype=np.float32)
    V_w = np.asarray(V_w, dtype=np.float32)

    q = query[0][:, -1, :]  # (B, H)
    v0t = np.ascontiguousarray(values[0].T.astype(np.float16))  # (H, S)
    qt = np.ascontiguousarray(q.T.astype(np.float16))  # (H, B)

    sel = np.zeros((B, 2), np.float16)
    for j in range(NC):
        sel[2 * j, 0] = 1.0
        sel[2 * j + 1, 1] = 1.0

    in_maps = []
    for i in range(NC):
        hsl = slice(HLOC * i, HLOC * (i + 1))
        w2t_i = np.ascontiguousarray(W2_w[hsl, :].T.astype(np.float16))  # (H, HLOC)
        w1t_i = np.ascontiguousarray(W1_w[hsl, :].T.astype(np.float16))
        b12_i = np.zeros((P, 2, 2), np.float32)
        b12_i[:, :, 0] = W1_b[hsl].reshape(2, P).T
        b12_i[:, :, 1] = W2_b[hsl].reshape(2, P).T
        vwl = V_w[hsl].astype(np.float16).reshape(2, P)  # [m, p]
        vwe_i = np.zeros((P, 2, B, B), np.float16)
        for bb in range(B):
            vwe_i[:, :, bb, bb] = vwl.T
        in_maps.append(
            {
                "v0t": v0t,
                "w2t": w2t_i,
                "w1t": w1t_i,
                "qt": qt,
                "b12": b12_i,
                "vwe": vwe_i,
                "sel": sel,
                "vals": np.ascontiguousarray(values[2 * i : 2 * i + 2].astype(np.float16)),
            }
        )

    nc = _get_module()
    res = run_bass_kernel_spmd(
        nc, in_maps, core_ids=list(range(NC)), trace=_TRACE
    )
    LAST_EXEC_NS = res.exec_time_ns

    ctx = np.concatenate([res.results[i]["ctx"] for i in range(NC)], axis=0)
    alps = np.concatenate([res.results[i]["alp"] for i in range(NC)], axis=0)
    return ctx.reshape(B, 1, H), alps.reshape(B, 1, S)


# revision 13
# speedup vs baseline: 1.4083x; 1.1070x over previous
"""Bahdanau attention kernel for 8 Trainium2 NeuronCores.

Strategy (single SPMD launch, one NEFF on all 8 cores):
  - Scores phase is tensor-parallel over the hidden dim H: core i owns
    h-slice [256*i, 256*(i+1)).  v_projT is computed in two s-halves so
    the ScalarE tanh pipeline starts as soon as the first half of
    values[0].T has streamed in.  Weights are fp16, DMA-ordered so v0t
    streams at full rate before the context values prefetch.
  - Partial scores are exchanged with an AllToAll (fp16, 64KB) instead
    of a ReduceScatter; the 8 shards are summed on the PE with a small
    selector matmul, in BOTH layouts: [2, S] for the alphas output and
    [S-partition, 2] (transposed) feeding exp directly into the fp16
    alphasT tile for the context matmul (no PE transposes needed).
  - exp without max-subtraction (scores are O(3)), normalization folded
    into the PSUM->SBUF copy of the context rows.
  - Context phase is 2-way column-tiled (tile_position): the two
    batches' accumulation chains run concurrently on separate PE column
    groups.  A junk-MM chain keeps the PE clock warm through the
    collective window.
Host side only reshapes/slices/transposes inputs (sharding layout) and
concatenates the per-core outputs.
"""

import sys

sys.path.insert(0, "/opt/trn_rl_repo")

import numpy as np

import concourse.bass as bass  # noqa: F401  (registers AP machinery)
import concourse.tile as tile
from concourse import bacc, mybir
from concourse.bass_utils import run_bass_kernel_spmd

H = 2048
B = 16
S = 2048
NC = 8
P = 128
HLOC = H // NC  # 256
KT = H // P  # 16 contraction tiles
SH = S // 2  # 1024: s-half size
NT = S // 512  # 4 free-dim slices of 512

F32 = mybir.dt.float32
F16 = mybir.dt.float16
BF16 = mybir.dt.bfloat16

N_PRE = 32  # context-values tiles prefetched (all of them)

_TRACE = False
LAST_EXEC_NS = None

_NC_CACHE = []


def _build_module():
    nc = bacc.Bacc("TRN2", target_bir_lowering=False, debug=False, num_devices=NC)

    v0t = nc.dram_tensor("v0t", [H, S], F16, kind="ExternalInput")  # values[0].T
    w2t = nc.dram_tensor("w2t", [H, HLOC], F16, kind="ExternalInput")  # W2[h_i].T
    w1t = nc.dram_tensor("w1t", [H, HLOC], F16, kind="ExternalInput")  # W1[h_i].T
    qt = nc.dram_tensor("qt", [H, B], F16, kind="ExternalInput")  # q.T
    b12 = nc.dram_tensor("b12", [P, 2, 2], F32, kind="ExternalInput")  # biases
    vwe = nc.dram_tensor("vwe", [P, 2, B, B], F16, kind="ExternalInput")
    sel = nc.dram_tensor("sel", [B, 2], F16, kind="ExternalInput")  # shard-sum sel
    vals = nc.dram_tensor("vals", [2, S, H], F16, kind="ExternalInput")
    ctx_o = nc.dram_tensor("ctx", [2, H], F32, kind="ExternalOutput")
    alp_o = nc.dram_tensor("alp", [2, S], F32, kind="ExternalOutput")

    with tile.TileContext(nc) as tc:
        with tc.tile_pool(name="const", bufs=1) as const:
            # ---- resident SBUF state -------------------------------------
            w2s = const.tile([P, KT, HLOC], F16)
            nc.sync.dma_start(
                out=w2s, in_=w2t[:, :].rearrange("(t p) m -> p t m", p=P)
            )
            vwes = const.tile([P, 2, B, B], F16)
            nc.gpsimd.dma_start(out=vwes, in_=vwe[:, :, :, :])
            b12s = const.tile([P, 2, 2], F32)
            nc.gpsimd.dma_start(out=b12s, in_=b12[:, :, :])
            sels = const.tile([B, 2], F16)
            nc.gpsimd.dma_start(out=sels, in_=sel[:, :])

            bsum = const.tile([P, 2], F32)
            nc.vector.tensor_add(out=bsum, in0=b12s[:, :, 0], in1=b12s[:, :, 1])

            qpt = const.tile([P, 2, B], F32)  # q_projT + bias
            vps = const.tile([P, 2, S], F32)  # v_projT (SBUF resident)
            scs = const.tile([B, S], F16)  # partial scores (fp16 for A2A)
            a2as = const.tile([B, S], F16)  # A2A result: 8 stacked shards
            alT = const.tile([P, KT, 2], F16)  # exp(scores) transposed
            alp = const.tile([2, S], F32)  # alphas ([2, S] output path)
            ssum = const.tile([2, 1], F32)
            rec = const.tile([2, 1], F32)
            rec32 = const.tile([33, 1], F32)  # rec[b] scattered to partition 32b
            ctxs = const.tile([33, H], F32)  # rows 0 and 32 used
            wu = const.tile([P, 512], BF16)  # PE warm-up junk

            with tc.tile_pool(name="pha", bufs=1) as pha:
                w1s = pha.tile([P, KT, HLOC], F16)
                nc.sync.dma_start(
                    out=w1s, in_=w1t[:, :].rearrange("(t p) m -> p t m", p=P)
                )
                qts = pha.tile([P, KT, B], F16)
                nc.sync.dma_start(
                    out=qts, in_=qt[:, :].rearrange("(t p) b -> p t b", p=P)
                )

                # ---- phase A: q_projT ------------------------------------
                with tc.tile_pool(name="psa", bufs=2, space="PSUM") as psa:
                    for m in range(2):
                        qp_ps = psa.tile([P, B], F32, tag="qp", name="qp")
                        for kt in range(KT):
                            nc.tensor.matmul(
                                qp_ps[:, :],
                                w1s[:, kt, m * P : (m + 1) * P],
                                qts[:, kt, :],
                                start=(kt == 0),
                                stop=(kt == KT - 1),
                            )
                        nc.vector.tensor_scalar_add(
                            out=qpt[:, m, :], in0=qp_ps[:, :],
                            scalar1=bsum[:, m : m + 1],
                        )

                # ---- PE warm-up: dummy matmuls ---------------------------
                nc.vector.memset(wu[:, :], 0.0)
                with tc.tile_pool(name="psw", bufs=1, space="PSUM") as psw:
                    wup = psw.tile([P, 512], F32, tag="wup", name="wup")
                    n_wu = 12
                    for i in range(n_wu):
                        nc.tensor.matmul(
                            wup[:, :], wu[:, 0:P], wu[:, :],
                            start=(i == 0), stop=(i == n_wu - 1),
                        )
                    nc.vector.tensor_copy(out=wu[:, 0:P], in_=wup[:, 0:P])

            # ---- phase B: v_projT (fp16), streamed in two s-halves -------
            # v0t is read in 1MB chunks (4 kt-tiles x one s-half) for DMA
            # efficiency.
            with (
                tc.tile_pool(name="psb", bufs=1, space="PSUM") as psb,
                tc.tile_pool(name="v0p", bufs=4) as v0p,
            ):
                for half in range(2):
                    vpp = [
                        [
                            psb.tile(
                                [P, 512], F32,
                                name=f"vp{m}{nt2}", tag=f"vp{m}{nt2}",
                            )
                            for nt2 in range(2)
                        ]
                        for m in range(2)
                    ]
                    for kt4 in range(KT // 4):
                        rv = v0p.tile([P, 4, SH], F16, tag="rv", name="rv")
                        nc.sync.dma_start(
                            out=rv,
                            in_=v0t[
                                kt4 * 512 : (kt4 + 1) * 512,
                                half * SH : (half + 1) * SH,
                            ].rearrange("(four p) s -> p four s", p=P),
                        )
                        for four in range(4):
                            kt = kt4 * 4 + four
                            for m in range(2):
                                for nt2 in range(2):
                                    nc.tensor.matmul(
                                        vpp[m][nt2][:, :],
                                        w2s[:, kt, m * P : (m + 1) * P],
                                        rv[:, four, nt2 * 512 : (nt2 + 1) * 512],
                                        start=(kt == 0),
                                        stop=(kt == KT - 1),
                                    )
                    for m in range(2):
                        for nt2 in range(2):
                            nc.vector.tensor_copy(
                                out=vps[
                                    :, m,
                                    half * SH + nt2 * 512 : half * SH + (nt2 + 1) * 512,
                                ],
                                in_=vpp[m][nt2],
                            )

            # context values: prefetch after v0t is fully queued
            vlp_cm = tc.tile_pool(name="vlp", bufs=N_PRE)
            vlp = vlp_cm.__enter__()
            vts = {}
            for j in range(N_PRE):
                b, kt = divmod(j, KT)
                vt = vlp.tile([P, H], F16, tag="vt", name="vt")
                nc.sync.dma_start(
                    out=vt, in_=vals[b, kt * P : (kt + 1) * P, :]
                )
                vts[(b, kt)] = vt

            # ---- phase C: tanh + partial scores (per s-half) -------------
            with (
                tc.tile_pool(name="psc", bufs=1, space="PSUM") as psc,
                tc.tile_pool(name="thp", bufs=12) as thp,
                tc.tile_pool(name="drp", bufs=1, space="DRAM") as drp,
            ):
                scps = psc.tile([B, S], F32, name="scps", tag="scps")
                arin = [
                    drp.tile([B, SH], F16, name=f"arin{h}") for h in range(2)
                ]
                arout = [
                    drp.tile([B, SH], F16, name=f"arout{h}") for h in range(2)
                ]
                for half in range(2):
                    for b in range(B):
                        for m in range(2):
                            th = thp.tile([P, SH], F16, tag="th", name="th")
                            nc.scalar.activation(
                                out=th[:, :],
                                in_=vps[:, m, half * SH : (half + 1) * SH],
                                func=mybir.ActivationFunctionType.Tanh,
                                bias=qpt[:, m, b : b + 1],
                                scale=1.0,
                            )
                            for nt2 in range(2):
                                nc.tensor.matmul(
                                    scps[
                                        :,
                                        half * SH + nt2 * 512 : half * SH + (nt2 + 1) * 512,
                                    ],
                                    vwes[:, m, b, :],
                                    th[:, nt2 * 512 : (nt2 + 1) * 512],
                                    start=(b == 0 and m == 0),
                                    stop=(b == B - 1 and m == 1),
                                )
                    # ---- phase D (per half): AllToAll of partial scores.
                    # The first A2A rides under tanh of the second half and
                    # absorbs the inter-core launch skew; the second one then
                    # runs close to its latency floor.  Writes + triggers go
                    # on the gpsimd queue so the vals prefetch stream (sync
                    # queue) can't head-of-line block them.
                    nc.vector.tensor_copy(
                        out=scs[:, half * SH : (half + 1) * SH],
                        in_=scps[:, half * SH : (half + 1) * SH],
                    )
                    nc.gpsimd.dma_start(
                        out=arin[half][:, :],
                        in_=scs[:, half * SH : (half + 1) * SH],
                    )
                    nc.gpsimd.collective_compute(
                        "AllToAll",
                        mybir.AluOpType.bypass,
                        replica_groups=[list(range(NC))],
                        ins=[arin[half].opt()],
                        outs=[arout[half].opt()],
                    )
                    nc.sync.dma_start(
                        out=a2as[:, half * SH : (half + 1) * SH],
                        in_=arout[half][:, :],
                    )

            # ---- phase E: shard-sum on PE + softmax ----------------------
            # transposed layout first (critical path -> context)
            with tc.tile_pool(name="psqt", bufs=1, space="PSUM") as psqt:
                scpT = psqt.tile([P, KT, 2], F32, name="scpT", tag="scpT")
                for j in range(KT):
                    nc.tensor.matmul(
                        scpT[:, j, :],
                        a2as[:, j * P : (j + 1) * P],
                        sels[:, :],
                        start=True,
                        stop=True,
                    )
                # unnormalized exp(scores).T straight into the fp16 tile
                nc.scalar.activation(
                    out=alT[:, :, :],
                    in_=scpT[:, :, :],
                    func=mybir.ActivationFunctionType.Exp,
                    scale=1.0,
                )

            with tc.tile_pool(name="psq2", bufs=1, space="PSUM") as psq2:
                scp2 = psq2.tile([2, S], F32, name="scp2", tag="scp2")
                for nt in range(NT):
                    nc.tensor.matmul(
                        scp2[:, nt * 512 : (nt + 1) * 512],
                        sels[:, :],
                        a2as[:, nt * 512 : (nt + 1) * 512],
                        start=True,
                        stop=True,
                    )
                # alphas output path (off the context critical path)
                nc.scalar.activation(
                    out=alp[:, :],
                    in_=scp2[:, :],
                    func=mybir.ActivationFunctionType.Exp,
                    scale=1.0,
                    accum_out=ssum[:, 0:1],
                )
                nc.vector.reciprocal(out=rec, in_=ssum)
                nc.gpsimd.dma_start(out=rec32[0:33:32, 0:1], in_=rec[:, 0:1])
                nc.vector.tensor_scalar_mul(
                    out=alp[:, :], in0=alp[:, :], scalar1=rec[:, 0:1]
                )
                nc.gpsimd.dma_start(out=alp_o[:, :], in_=alp[:, :])

                # ---- phase G: context, 2-way column-tiled ----------------
                with tc.tile_pool(name="psg", bufs=1, space="PSUM") as psg:
                    cps = psg.tile([P, S], F32, name="cps", tag="cps")
                    for kt in range(KT):
                        for b in range(2):
                            vt = vts[(b, kt)]
                            for nt in range(NT):
                                nc.tensor.matmul(
                                    cps[32 * b : 32 * b + 1, nt * 512 : (nt + 1) * 512],
                                    alT[:, kt, b : b + 1],
                                    vt[:, nt * 512 : (nt + 1) * 512],
                                    tile_position=(0, 32 * b),
                                    start=(kt == 0),
                                    stop=(kt == KT - 1),
                                )
                    # normalize while copying PSUM -> SBUF (DVE + ScalarE in
                    # parallel, one context row each)
                    nc.vector.tensor_scalar_mul(
                        out=ctxs[0:1, :],
                        in0=cps[0:1, :],
                        scalar1=rec32[0:1, 0:1],
                    )
                    nc.scalar.activation(
                        out=ctxs[32:33, :],
                        in_=cps[32:33, :],
                        func=mybir.ActivationFunctionType.Copy,
                        scale=rec32[32:33, 0:1],
                    )
                    nc.sync.dma_start(out=ctx_o[:, :], in_=ctxs[0:33:32, :])
            vlp_cm.__exit__(None, None, None)

    nc.compile()
    return nc


def _get_module():
    if not _NC_CACHE:
        _NC_CACHE.append(_build_module())
    return _NC_CACHE[0]


def kernel(query, values, mask=None, W1_w=None, W1_b=None, W2_w=None, W2_b=None,
           V_w=None, V_b=None):
    global LAST_EXEC_NS
    query = np.ascontiguousarray(np.asarray(query, dtype=np.float32))
    values = np.ascontiguousarray(np.asarray(values, dtype=np.float32))
    W1_w = np.asarray(W1_w, dtype=np.float32)
    W1_b = np.asarray(W1_b, dtype=np.float32)
    W2_w = np.asarray(W2_w, dtype=np.float32)
    W2_b = np.asarray(W2_b, dtype=np.float32)
    V_w = np.asarray(V_w, dtype=np.float32)

    q = query[0][:, -1, :]  # (B, H)
    v0t = np.ascontiguousarray(values[0].T.astype(np.float16))  # (H, S)
    qt = np.ascontiguousarray(q.T.astype(np.float16))  # (H, B)

    sel = np.zeros((B, 2), np.float16)
    for j in range(NC):
        sel[2 * j, 0] = 1.0
        sel[2 * j + 1, 1] = 1.0

    in_maps = []
    for i in range(NC):
        hsl = slice(HLOC * i, HLOC * (i + 1))
        w2t_i = np.ascontiguousarray(W2_w[hsl, :].T.astype(np.float16))  # (H, HLOC)
        w1t_i = np.ascontiguousarray(W1_w[hsl, :].T.astype(np.float16))
        b12_i = np.zeros((P, 2, 2), np.float32)
        b12_i[:, :, 0] = W1_b[hsl].reshape(2, P).T
        b12_i[:, :, 1] = W2_b[hsl].reshape(2, P).T
        vwl = V_w[hsl].astype(np.float16).reshape(2, P)  # [m, p]
        vwe_i = np.zeros((P, 2, B, B), np.float16)
        for bb in range(B):
            vwe_i[:, :, bb, bb] = vwl.T
        in_maps.append(
            {
                "v0t": v0t,
                "w2t": w2t_i,
                "w1t": w1t_i,
                "qt": qt,
                "b12": b12_i,
                "vwe": vwe_i,
                "sel": sel,
                "vals": np.ascontiguousarray(values[2 * i : 2 * i + 2].astype(np.float16)),
            }
        )

    nc = _get_module()
    res = run_bass_kernel_spmd(
        nc, in_maps, core_ids=list(range(NC)), trace=_TRACE
    )
    LAST_EXEC_NS = res.exec_time_ns

    ctx = np.concatenate([res.results[i]["ctx"] for i in range(NC)], axis=0)
    alps = np.concatenate([res.results[i]["alp"] for i in range(NC)], axis=0)
    return ctx.reshape(B, 1, H), alps.reshape(B, 1, S)


# revision 15
# speedup vs baseline: 1.4421x; 1.0239x over previous
"""Bahdanau attention kernel for 8 Trainium2 NeuronCores.

Strategy (single SPMD launch, one NEFF on all 8 cores):
  - Scores phase is tensor-parallel over the hidden dim H: core i owns
    h-slice [256*i, 256*(i+1)).  v_projT is computed in two asymmetric
    s-chunks (768 / 1280) so the ScalarE tanh pipeline starts as soon as
    the first 768 columns of values[0].T have streamed in (1.25-1.5MB
    DMA chunks, weights host-pretransposed for contiguous loads).
  - Partial scores are exchanged with two AllToAll collectives (fp16),
    one per s-chunk.  The first one is triggered mid-tanh and absorbs
    the collective stack's large first-op latency / inter-core skew;
    the second then runs near its floor.  Shards are summed on the PE
    with a small selector matmul, in BOTH layouts: [2, S] for the
    alphas output and [S-partition, 2] (transposed) feeding exp
    directly into the fp16 alphasT tile for the context matmul (no PE
    transposes).  exp without max-subtraction (scores are O(3)).
  - Context phase is 2-way column-tiled (tile_position): the two
    batches' accumulation chains run concurrently on separate PE column
    groups.  The first 6 of 16 kt-tiles of context run hidden under the
    second collective's window; normalization is folded into the
    PSUM->SBUF copies (DVE + ScalarE in parallel).
  - Queue routing: bulk streams on sync, collective input writes on
    gpsimd/scalar, triggers on gpsimd, so nothing head-of-line blocks.
Host side only reshapes/slices/transposes inputs (sharding layout) and
concatenates the per-core outputs.
"""

import sys

sys.path.insert(0, "/opt/trn_rl_repo")

import numpy as np

import concourse.bass as bass  # noqa: F401  (registers AP machinery)
import concourse.tile as tile
from concourse import bacc, mybir
from concourse.bass_utils import run_bass_kernel_spmd

H = 2048
B = 16
S = 2048
NC = 8
P = 128
HLOC = H // NC  # 256
KT = H // P  # 16 contraction tiles
NT = S // 512  # 4 free-dim slices of 512

S0 = 768  # first s-chunk (tanh starts early on this)
S1 = S - S0  # 1280
KT0 = S0 // P  # 6: kt tiles of context covered by the first AllToAll

F32 = mybir.dt.float32
F16 = mybir.dt.float16
BF16 = mybir.dt.bfloat16

N_PRE = 28  # context-values tiles prefetched up front

_TRACE = False
LAST_EXEC_NS = None

_NC_CACHE = []


def _nsplit(width):
    """Split a row of `width` fp32 into <=512-wide matmul column chunks."""
    out = []
    c = 0
    while c < width:
        w = min(512, width - c)
        out.append((c, w))
        c += w
    return out


def _build_module():
    nc = bacc.Bacc("TRN2", target_bir_lowering=False, debug=False, num_devices=NC)

    v0t = nc.dram_tensor("v0t", [H, S], F16, kind="ExternalInput")  # values[0].T
    w2s_h = nc.dram_tensor("w2s_h", [P, KT, HLOC], F16, kind="ExternalInput")
    w1s_h = nc.dram_tensor("w1s_h", [P, KT, HLOC], F16, kind="ExternalInput")
    qts_h = nc.dram_tensor("qts_h", [P, KT, B], F16, kind="ExternalInput")
    b12 = nc.dram_tensor("b12", [P, 2, 2], F32, kind="ExternalInput")  # biases
    vwe = nc.dram_tensor("vwe", [P, 2, B, B], F16, kind="ExternalInput")
    sel = nc.dram_tensor("sel", [B, 2], F16, kind="ExternalInput")  # shard-sum sel
    vals = nc.dram_tensor("vals", [2, S, H], F16, kind="ExternalInput")
    ctx_o = nc.dram_tensor("ctx", [2, H], F32, kind="ExternalOutput")
    alp_o = nc.dram_tensor("alp", [2, S], F32, kind="ExternalOutput")

    with tile.TileContext(nc) as tc:
        with tc.tile_pool(name="const", bufs=1) as const:
            # ---- resident SBUF state -------------------------------------
            w2s = const.tile([P, KT, HLOC], F16)
            nc.sync.dma_start(out=w2s, in_=w2s_h[:, :, :])
            vwes = const.tile([P, 2, B, B], F16)
            nc.gpsimd.dma_start(out=vwes, in_=vwe[:, :, :, :])
            b12s = const.tile([P, 2, 2], F32)
            nc.gpsimd.dma_start(out=b12s, in_=b12[:, :, :])
            sels = const.tile([B, 2], F16)
            nc.gpsimd.dma_start(out=sels, in_=sel[:, :])

            bsum = const.tile([P, 2], F32)
            nc.vector.tensor_add(out=bsum, in0=b12s[:, :, 0], in1=b12s[:, :, 1])

            qpt = const.tile([P, 2, B], F32)  # q_projT + bias
            vps = const.tile([P, 2, S], F32)  # v_projT (SBUF resident)
            scs = const.tile([B, S], F16)  # partial scores (fp16 for A2A)
            a2as = const.tile([B, S], F16)  # A2A result: 8 stacked shards
            alT = const.tile([P, KT, 2], F16)  # exp(scores) transposed
            alp = const.tile([2, S], F32)  # alphas ([2, S] output path)
            ssum = const.tile([2, 1], F32)
            rec = const.tile([2, 1], F32)
            rec32 = const.tile([33, 1], F32)  # rec[b] scattered to partition 32b
            ctxs = const.tile([33, H], F32)  # rows 0 and 32 used

            with tc.tile_pool(name="pha", bufs=1) as pha:
                w1s = pha.tile([P, KT, HLOC], F16)
                nc.sync.dma_start(out=w1s, in_=w1s_h[:, :, :])
                qts = pha.tile([P, KT, B], F16)
                nc.sync.dma_start(out=qts, in_=qts_h[:, :, :])

                # ---- phase A: q_projT ------------------------------------
                with tc.tile_pool(name="psa", bufs=2, space="PSUM") as psa:
                    for m in range(2):
                        qp_ps = psa.tile([P, B], F32, tag="qp", name="qp")
                        for kt in range(KT):
                            nc.tensor.matmul(
                                qp_ps[:, :],
                                w1s[:, kt, m * P : (m + 1) * P],
                                qts[:, kt, :],
                                start=(kt == 0),
                                stop=(kt == KT - 1),
                            )
                        nc.vector.tensor_scalar_add(
                            out=qpt[:, m, :], in0=qp_ps[:, :],
                            scalar1=bsum[:, m : m + 1],
                        )

            # ---- phase B: v_projT (fp16), two asymmetric s-chunks --------
            # chunk 0: s[0:768] via 2 DMAs of [P, 8, 768] (1.5MB each)
            # chunk 1: s[768:2048] via 4 DMAs of [P, 4, 1280] (1.25MB each)
            chunk_cfg = [
                (0, S0, 2, 8),  # (s_off, s_width, n_dma, kt_per_dma)
                (S0, S1, 4, 4),
            ]
            with tc.tile_pool(name="v0p", bufs=3) as v0p:
                for half, (soff, swid, ndma, ktpd) in enumerate(chunk_cfg):
                  with tc.tile_pool(name=f"psb{half}", bufs=1, space="PSUM") as psb:
                    vpp = [
                        psb.tile([P, swid], F32, name=f"vp{m}", tag=f"vph{half}{m}")
                        for m in range(2)
                    ]
                    for d in range(ndma):
                        rv = v0p.tile([P, ktpd, swid], F16, tag="rv", name="rv")
                        nc.sync.dma_start(
                            out=rv,
                            in_=v0t[
                                d * ktpd * P : (d + 1) * ktpd * P,
                                soff : soff + swid,
                            ].rearrange("(g p) s -> p g s", p=P),
                        )
                        for g in range(ktpd):
                            kt = d * ktpd + g
                            for m in range(2):
                                for c, w in _nsplit(swid):
                                    nc.tensor.matmul(
                                        vpp[m][:, c : c + w],
                                        w2s[:, kt, m * P : (m + 1) * P],
                                        rv[:, g, c : c + w],
                                        start=(kt == 0),
                                        stop=(kt == KT - 1),
                                    )
                    for m in range(2):
                        nc.vector.tensor_copy(
                            out=vps[:, m, soff : soff + swid], in_=vpp[m][:, :]
                        )

            # context values: prefetch after v0t is fully queued
            vlp_cm = tc.tile_pool(name="vlp", bufs=N_PRE)
            vlp = vlp_cm.__enter__()
            vts = {}
            for j in range(N_PRE):
                b, kt = divmod(j, KT)
                vt = vlp.tile([P, H], F16, tag="vt", name="vt")
                nc.sync.dma_start(
                    out=vt, in_=vals[b, kt * P : (kt + 1) * P, :]
                )
                vts[(b, kt)] = vt

            # ---- phase C + D: tanh, partial scores, per-chunk AllToAll ---
            with (
                tc.tile_pool(name="drp", bufs=1, space="DRAM") as drp,
                tc.tile_pool(name="thp0", bufs=24) as thp0,
                tc.tile_pool(name="thp1", bufs=4) as thp1,
            ):
                arin = [
                    drp.tile([B, S0], F16, name="arin0"),
                    drp.tile([B, S1], F16, name="arin1"),
                ]
                arout = [
                    drp.tile([B, S0], F16, name="arout0"),
                    drp.tile([B, S1], F16, name="arout1"),
                ]
                for half, (soff, swid, _, _) in enumerate(chunk_cfg):
                    thp = thp0 if half == 0 else thp1
                    with tc.tile_pool(name=f"psc{half}", bufs=1, space="PSUM") as psc:
                        scps = psc.tile(
                            [B, swid], F32, name=f"scps{half}", tag=f"scps{half}"
                        )
                        for b in range(B):
                            for m in range(2):
                                th = thp.tile([P, swid], F16, tag="th", name="th")
                                nc.scalar.activation(
                                    out=th[:, :],
                                    in_=vps[:, m, soff : soff + swid],
                                    func=mybir.ActivationFunctionType.Tanh,
                                    bias=qpt[:, m, b : b + 1],
                                    scale=1.0,
                                )
                                for c, w in _nsplit(swid):
                                    nc.tensor.matmul(
                                        scps[:, c : c + w],
                                        vwes[:, m, b, :],
                                        th[:, c : c + w],
                                        start=(b == 0 and m == 0),
                                        stop=(b == B - 1 and m == 1),
                                    )
                        nc.vector.tensor_copy(
                            out=scs[:, soff : soff + swid], in_=scps[:, :]
                        )
                    # input write: gpsimd for chunk 0, scalar for chunk 1
                    # (so it isn't queued behind the blocking first trigger)
                    if half == 0:
                        nc.gpsimd.dma_start(
                            out=arin[half][:, :], in_=scs[:, soff : soff + swid]
                        )
                    else:
                        nc.scalar.dma_start(
                            out=arin[half][:, :], in_=scs[:, soff : soff + swid]
                        )
                    nc.gpsimd.collective_compute(
                        "AllToAll",
                        mybir.AluOpType.bypass,
                        replica_groups=[list(range(NC))],
                        ins=[arin[half].opt()],
                        outs=[arout[half].opt()],
                    )
                    nc.sync.dma_start(
                        out=a2as[:, soff : soff + swid], in_=arout[half][:, :]
                    )

            # ---- phase E/F/G: shard-sum, softmax, context ----------------
            with tc.tile_pool(name="psg", bufs=1, space="PSUM") as psg:
                cps = psg.tile([P, S], F32, name="cps", tag="cps")

                def context_mms(kt_lo, kt_hi):
                    for kt in range(kt_lo, kt_hi):
                        for b in range(2):
                            vt = vts.get((b, kt))
                            if vt is None:
                                vt = vlp.tile([P, H], F16, tag="vt", name="vt")
                                nc.sync.dma_start(
                                    out=vt,
                                    in_=vals[b, kt * P : (kt + 1) * P, :],
                                )
                                vts[(b, kt)] = vt
                            for nt in range(NT):
                                nc.tensor.matmul(
                                    cps[32 * b : 32 * b + 1, nt * 512 : (nt + 1) * 512],
                                    alT[:, kt, b : b + 1],
                                    vt[:, nt * 512 : (nt + 1) * 512],
                                    tile_position=(0, 32 * b),
                                    start=(kt == 0),
                                    stop=(kt == KT - 1),
                                )

                with tc.tile_pool(name="psqt", bufs=1, space="PSUM") as psqt:
                    scpT = psqt.tile([P, KT, 2], F32, name="scpT", tag="scpT")
                    for j in range(KT0):
                        nc.tensor.matmul(
                            scpT[:, j, :],
                            a2as[:, j * P : (j + 1) * P],
                            sels[:, :],
                            start=True,
                            stop=True,
                        )
                    nc.scalar.activation(
                        out=alT[:, 0:KT0, :],
                        in_=scpT[:, 0:KT0, :],
                        func=mybir.ActivationFunctionType.Exp,
                        scale=1.0,
                    )
                    # first 6 kt of context run hidden under the second A2A
                    context_mms(0, KT0)
                    for j in range(KT0, KT):
                        nc.tensor.matmul(
                            scpT[:, j, :],
                            a2as[:, j * P : (j + 1) * P],
                            sels[:, :],
                            start=True,
                            stop=True,
                        )
                    nc.scalar.activation(
                        out=alT[:, KT0:KT, :],
                        in_=scpT[:, KT0:KT, :],
                        func=mybir.ActivationFunctionType.Exp,
                        scale=1.0,
                    )

                with tc.tile_pool(name="psq2", bufs=1, space="PSUM") as psq2:
                    scp2 = psq2.tile([2, S], F32, name="scp2", tag="scp2")
                    for nt in range(NT):
                        nc.tensor.matmul(
                            scp2[:, nt * 512 : (nt + 1) * 512],
                            sels[:, :],
                            a2as[:, nt * 512 : (nt + 1) * 512],
                            start=True,
                            stop=True,
                        )
                    # alphas output path (off the context critical path)
                    nc.scalar.activation(
                        out=alp[:, :],
                        in_=scp2[:, :],
                        func=mybir.ActivationFunctionType.Exp,
                        scale=1.0,
                        accum_out=ssum[:, 0:1],
                    )
                    nc.vector.reciprocal(out=rec, in_=ssum)
                    nc.gpsimd.dma_start(out=rec32[0:33:32, 0:1], in_=rec[:, 0:1])
                    nc.vector.tensor_scalar_mul(
                        out=alp[:, :], in0=alp[:, :], scalar1=rec[:, 0:1]
                    )
                    nc.gpsimd.dma_start(out=alp_o[:, :], in_=alp[:, :])

                    # rest of the context
                    context_mms(KT0, KT)

                    # normalize while copying PSUM -> SBUF (DVE + ScalarE in
                    # parallel, one context row each)
                    nc.vector.tensor_scalar_mul(
                        out=ctxs[0:1, :],
                        in0=cps[0:1, :],
                        scalar1=rec32[0:1, 0:1],
                    )
                    nc.scalar.activation(
                        out=ctxs[32:33, :],
                        in_=cps[32:33, :],
                        func=mybir.ActivationFunctionType.Copy,
                        scale=rec32[32:33, 0:1],
                    )
                    nc.sync.dma_start(out=ctx_o[:, :], in_=ctxs[0:33:32, :])
            vlp_cm.__exit__(None, None, None)

    nc.compile()
    return nc


def _get_module():
    if not _NC_CACHE:
        _NC_CACHE.append(_build_module())
    return _NC_CACHE[0]


def kernel(query, values, mask=None, W1_w=None, W1_b=None, W2_w=None, W2_b=None,
           V_w=None, V_b=None):
    global LAST_EXEC_NS
    query = np.ascontiguousarray(np.asarray(query, dtype=np.float32))
    values = np.ascontiguousarray(np.asarray(values, dtype=np.float32))
    W1_w = np.asarray(W1_w, dtype=np.float32)
    W1_b = np.asarray(W1_b, dtype=np.float32)
    W2_w = np.asarray(W2_w, dtype=np.float32)
    W2_b = np.asarray(W2_b, dtype=np.float32)
    V_w = np.asarray(V_w, dtype=np.float32)

    q = query[0][:, -1, :]  # (B, H)
    v0t = np.ascontiguousarray(values[0].T.astype(np.float16))  # (H, S)
    qt = q.T.astype(np.float16)  # (H, B)
    # device layout [P, KT, B]: partition p, tile kt -> h = kt*128 + p
    qts_h = np.ascontiguousarray(qt.reshape(KT, P, B).transpose(1, 0, 2))

    sel = np.zeros((B, 2), np.float16)
    for j in range(NC):
        sel[2 * j, 0] = 1.0
        sel[2 * j + 1, 1] = 1.0

    in_maps = []
    for i in range(NC):
        hsl = slice(HLOC * i, HLOC * (i + 1))
        w2t_i = W2_w[hsl, :].T.astype(np.float16)  # (H, HLOC)
        w1t_i = W1_w[hsl, :].T.astype(np.float16)
        w2s_i = np.ascontiguousarray(w2t_i.reshape(KT, P, HLOC).transpose(1, 0, 2))
        w1s_i = np.ascontiguousarray(w1t_i.reshape(KT, P, HLOC).transpose(1, 0, 2))
        b12_i = np.zeros((P, 2, 2), np.float32)
        b12_i[:, :, 0] = W1_b[hsl].reshape(2, P).T
        b12_i[:, :, 1] = W2_b[hsl].reshape(2, P).T
        vwl = V_w[hsl].astype(np.float16).reshape(2, P)  # [m, p]
        vwe_i = np.zeros((P, 2, B, B), np.float16)
        for bb in range(B):
            vwe_i[:, :, bb, bb] = vwl.T
        in_maps.append(
            {
                "v0t": v0t,
                "w2s_h": w2s_i,
                "w1s_h": w1s_i,
                "qts_h": qts_h,
                "b12": b12_i,
                "vwe": vwe_i,
                "sel": sel,
                "vals": np.ascontiguousarray(values[2 * i : 2 * i + 2].astype(np.float16)),
            }
        )

    nc = _get_module()
    res = run_bass_kernel_spmd(
        nc, in_maps, core_ids=list(range(NC)), trace=_TRACE
    )
    LAST_EXEC_NS = res.exec_time_ns

    ctx = np.concatenate([res.results[i]["ctx"] for i in range(NC)], axis=0)
    alps = np.concatenate([res.results[i]["alp"] for i in range(NC)], axis=0)
    return ctx.reshape(B, 1, H), alps.reshape(B, 1, S)
